# revision 18
# baseline (speedup 1.0000x reference)
"""VQ codebook (vector-quantization) kernel for Trainium2, 8 NeuronCores.

Math (per the nn.Module reference):
    flat = inputs.reshape(-1, 512)                    # (32768, 512)
    d[n,k] = ||flat_n||^2 + ||E_k||^2 - 2 flat_n . E_k
    idx    = argmin_k d  (first index wins ties)
    q      = E[idx]
    qst    = flat + (q - flat)          # straight-through, fp32 rounding kept
    loss   = 0.25 * mean((q - flat)^2)

Sharding: data-parallel over B=8 (one batch row of 4096 tokens per core),
codebook replicated. The scalar loss is reduced on the host from per-core
partial sums (the only cross-core term).

Numerical strategy (required to match the fp32 CPU reference's argmin,
including its ~75 exact fp32 distance ties):
  - the 2*x.E term is computed as a 3-pass bf16-split matmul
    (hi*hi + hi*lo + lo*hi of x and 2E^T) accumulated in fp32 PSUM;
    error vs a true fp32 matmul is ~1e-8, far below the reference's own
    distance-grid quantization (ulp(~512) = 6.1e-5).
  - the reference's rounding structure fl(fl(A+B) - C) is replicated
    exactly: A=||x||^2 per row, B=||E||^2 per code, C=2*x.E.
  - argmin via max8/max_index on the negated key (C - AB), which returns
    the first (lowest) index among ties, matching jnp.argmin.
"""

import numpy as np
import ml_dtypes

import concourse.bacc as bacc
import concourse.bass as bass
import concourse.mybir as mybir
import concourse.tile as tile
from concourse.bass_utils import run_bass_kernel_spmd
from concourse.masks import make_identity

NCORES = 8
B_DIM, T_DIM, D, K = 8, 4096, 512, 1024
N = T_DIM            # rows per core
P = 128              # partitions
NT = N // P          # 32 row-tiles per core
NC_CHUNKS = D // P   # 4 contraction chunks
KH = K // 512        # 2 psum halves

F32 = mybir.dt.float32
BF16 = mybir.dt.bfloat16
U32 = mybir.dt.uint32

COMMIT_COST = 0.25

_CACHE = {}


def build_nc():
    # Bacc (not bass.Bass): its compile() legalizes sync waits — walrus
    # accepts at most one wait per instruction, and Bacc's
    # generate_event_semaphores/move_matmul_waits_to_ldweights passes
    # split excess waits onto standalone event-semaphore instructions.
    nc = bacc.Bacc("TRN2")

    x_d = nc.dram_tensor("x", [N, D], F32, kind="ExternalInput")
    e_d = nc.dram_tensor("E", [K, D], F32, kind="ExternalInput")
    # hi and lo bf16 halves of (2E)^T stacked: [2*D, K]
    et2_d = nc.dram_tensor("Et2", [2 * D, K], BF16, kind="ExternalInput")
    brep_d = nc.dram_tensor("Brep", [P, K], F32, kind="ExternalInput")

    qst_d = nc.dram_tensor("qst", [N, D], F32, kind="ExternalOutput")
    idx_d = nc.dram_tensor("idx", [P, NT], U32, kind="ExternalOutput")
    lpart_d = nc.dram_tensor("lpart", [P, NT], F32, kind="ExternalOutput")

    with tile.TileContext(nc) as tc:
        with (
            tc.tile_pool(name="persist", bufs=1) as pp,
            tc.tile_pool(name="work", bufs=3) as wp,
            tc.tile_pool(name="big", bufs=3) as bp,
            tc.tile_pool(name="ps", bufs=2, space="PSUM") as psp,
        ):
            ident = pp.tile([P, P], F32)
            make_identity(nc, ident[:])

            # One DMA for the whole split codebook keeps the matmuls' sync
            # wait count within ISA limits (one DMA completion semaphore).
            et_all = pp.tile([P, 2 * NC_CHUNKS, K], BF16)
            nc.sync.dma_start(
                et_all[:], et2_d.rearrange("(g c p) k -> p (g c) k", g=2, p=P))
            ethi = [et_all[:, c, :] for c in range(NC_CHUNKS)]
            etlo = [et_all[:, NC_CHUNKS + c, :] for c in range(NC_CHUNKS)]
            brep = pp.tile([P, K], F32)
            nc.sync.dma_start(brep[:], brep_d[:])

            idx_stage = pp.tile([P, NT], U32)
            lpart = pp.tile([P, NT], F32)

            for t in range(NT):
                r0 = t * P
                # ---- load x tile [128 rows, 512 d] ----
                xt = wp.tile([P, D], F32, tag="xt")
                nc.sync.dma_start(xt[:], x_d[r0:r0 + P, :])

                # ---- transpose to [d, rows] via PE (4 chunks) ----
                xT_ps = psp.tile([P, D], F32, tag="xT")
                for c in range(NC_CHUNKS):
                    nc.tensor.transpose(
                        xT_ps[:, c * P:(c + 1) * P],
                        xt[:, c * P:(c + 1) * P],
                        ident[:],
                    )

                # ---- bf16 split of x^T (hi + lo) ----
                xT_hi = wp.tile([P, D], BF16, tag="xThi")
                xT_lo = wp.tile([P, D], BF16, tag="xTlo")
                nc.scalar.copy(xT_hi[:], xT_ps[:])
                nc.vector.tensor_tensor(
                    out=xT_lo[:], in0=xT_ps[:], in1=xT_hi[:],
                    op=mybir.AluOpType.subtract,
                )

                # ---- A = sum(x^2) per row (ACT Square + accumulate) ----
                sq = wp.tile([P, D], F32, tag="sq")
                a_col = wp.tile([P, 1], F32, tag="acol")
                nc.scalar.activation(
                    sq[:], xt[:], mybir.ActivationFunctionType.Square,
                    accum_out=a_col[:],
                )

                # ---- C = 2 x . E^T : 3-pass bf16 split matmul ----
                c_ps = psp.tile([P, K], F32, tag="C")
                for kh in range(KH):
                    mms = []
                    for lhs, rhs in ((xT_hi, ethi), (xT_hi, etlo), (xT_lo, ethi)):
                        for c in range(NC_CHUNKS):
                            mms.append((lhs, rhs[c], c))
                    for i, (lhs, rhs_t, c) in enumerate(mms):
                        nc.tensor.matmul(
                            c_ps[:, kh * 512:(kh + 1) * 512],
                            lhsT=lhs[:, c * P:(c + 1) * P],
                            rhs=rhs_t[:, kh * 512:(kh + 1) * 512],
                            start=(i == 0),
                            stop=(i == len(mms) - 1),
                        )

                # ---- AB = fl(B + A) broadcast (gpsimd, SBUF only) ----
                ab = bp.tile([P, K], F32, tag="ab")
                nc.gpsimd.tensor_scalar_add(ab[:], brep[:], a_col[:])

                # ---- key = fl(C - AB)  (= -dist, shifted) ----
                key = bp.tile([P, K], F32, tag="key")
                nc.vector.tensor_tensor(
                    out=key[:], in0=c_ps[:], in1=ab[:],
                    op=mybir.AluOpType.subtract,
                )

                # ---- argmax (first index on ties) ----
                mx = wp.tile([P, 8], F32, tag="mx")
                mi = wp.tile([P, 8], U32, tag="mi")
                nc.vector.max(out=mx[:], in_=key[:])
                nc.vector.max_index(out=mi[:], in_max=mx[:], in_values=key[:])
                nc.vector.tensor_copy(idx_stage[:, t:t + 1], mi[:, 0:1])

                # ---- gather q = E[idx] ----
                q = wp.tile([P, D], F32, tag="q")
                nc.gpsimd.indirect_dma_start(
                    out=q[:],
                    out_offset=None,
                    in_=e_d[:],
                    in_offset=bass.IndirectOffsetOnAxis(ap=mi[:, 0:1], axis=0),
                )

                # ---- diff = q - x ; qst = x + diff ; loss partial ----
                diff = wp.tile([P, D], F32, tag="diff")
                nc.gpsimd.tensor_tensor(
                    out=diff[:], in0=q[:], in1=xt[:],
                    op=mybir.AluOpType.subtract,
                )
                sq2 = wp.tile([P, D], F32, tag="sq2")
                nc.scalar.activation(
                    sq2[:], diff[:], mybir.ActivationFunctionType.Square,
                    accum_out=lpart[:, t:t + 1],
                )
                qst = wp.tile([P, D], F32, tag="qst")
                nc.gpsimd.tensor_tensor(
                    out=qst[:], in0=xt[:], in1=diff[:],
                    op=mybir.AluOpType.add,
                )
                nc.sync.dma_start(qst_d[r0:r0 + P, :], qst[:])

            nc.sync.dma_start(idx_d[:], idx_stage[:])
            nc.sync.dma_start(lpart_d[:], lpart[:])

    nc.compile()
    return nc


def _host_prep(inputs, embed_w):
    x = np.ascontiguousarray(np.asarray(inputs, dtype=np.float32))
    E = np.ascontiguousarray(np.asarray(embed_w, dtype=np.float32))
    assert x.shape == (B_DIM, T_DIM, D) and E.shape == (K, D)

    E2t = np.ascontiguousarray((2.0 * E).T)           # (512, 1024) fp32
    EtHi = E2t.astype(ml_dtypes.bfloat16)
    EtLo = (E2t - EtHi.astype(np.float32)).astype(ml_dtypes.bfloat16)
    Et2 = np.ascontiguousarray(np.concatenate([EtHi, EtLo], axis=0))
    Bsq = np.sum(E * E, axis=1, dtype=np.float32)     # (1024,)
    Brep = np.ascontiguousarray(np.broadcast_to(Bsq, (P, K)))
    return x, E, Et2, Brep


def kernel(inputs, embed_w, _trace=False):
    x, E, Et2, Brep = _host_prep(inputs, embed_w)

    if "nc" not in _CACHE:
        _CACHE["nc"] = build_nc()
    nc = _CACHE["nc"]

    in_maps = []
    for c in range(NCORES):
        in_maps.append({
            "x": np.ascontiguousarray(x[c]),
            "E": E,
            "Et2": Et2,
            "Brep": Brep,
        })

    if _trace:
        try:
            res = run_bass_kernel_spmd(
                nc, in_maps, core_ids=list(range(NCORES)), trace=True,
            )
        except Exception as e:  # tracing infra missing -> plain run
            print(f"trace run failed ({type(e).__name__}: {e}); rerunning untraced")
            res = run_bass_kernel_spmd(nc, in_maps, core_ids=list(range(NCORES)))
    else:
        res = run_bass_kernel_spmd(nc, in_maps, core_ids=list(range(NCORES)))

    qst = np.stack([res.results[c]["qst"] for c in range(NCORES)])
    qst = qst.reshape(B_DIM, T_DIM, D)

    # idx staged as [128 partitions, 32 tiles]: row 128*t + p  ->  arr[p, t]
    idx = np.stack([
        np.ascontiguousarray(res.results[c]["idx"].T).reshape(-1)
        for c in range(NCORES)
    ]).astype(np.int32)

    total = np.float64(0.0)
    for c in range(NCORES):
        total += res.results[c]["lpart"].sum(dtype=np.float64)
    loss = np.float32(COMMIT_COST * total / (B_DIM * T_DIM * D))

    if _trace:
        _CACHE["last_result"] = res
    return qst, idx, loss


# revision 20
# speedup vs baseline: 2.3352x; 2.3352x over previous
"""VQ codebook (vector-quantization) kernel for Trainium2, 8 NeuronCores.

Math (per the nn.Module reference):
    flat = inputs.reshape(-1, 512)                    # (32768, 512)
    d[n,k] = ||flat_n||^2 + ||E_k||^2 - 2 flat_n . E_k
    idx    = argmin_k d  (first index wins ties)
    q      = E[idx]
    qst    = flat + (q - flat)          # straight-through, fp32 rounding kept
    loss   = 0.25 * mean((q - flat)^2)

Sharding: data-parallel over B=8 (one batch row of 4096 tokens per core),
codebook replicated. The scalar loss is reduced on the host from per-core
partial sums (the only cross-core term).

Numerical strategy (required to match the fp32 CPU reference's argmin,
including its ~75 exact fp32 distance ties):
  - the 2*x.E term is computed as a 3-pass bf16-split matmul
    (hi*hi + hi*lo + lo*hi of x and 2E^T) accumulated in fp32 PSUM;
    error vs a true fp32 matmul is ~1e-8, far below the reference's own
    distance-grid quantization (ulp(~512) = 6.1e-5).
  - the reference's rounding structure fl(fl(A+B) - C) is replicated
    exactly: A=||x||^2 per row, B=||E||^2 per code, C=2*x.E.
  - argmin via max8/max_index on the negated key (C - AB), which returns
    the first (lowest) index among ties, matching jnp.argmin.
"""

import numpy as np
import ml_dtypes

import concourse.bacc as bacc
import concourse.bass as bass
import concourse.mybir as mybir
import concourse.tile as tile
from concourse.bass_utils import run_bass_kernel_spmd
from concourse.masks import make_identity

NCORES = 8
B_DIM, T_DIM, D, K = 8, 4096, 512, 1024
N = T_DIM            # rows per core
P = 128              # partitions
NT = N // P          # 32 row-tiles per core
NC_CHUNKS = D // P   # 4 contraction chunks
KH = K // 512        # 2 psum halves

F32 = mybir.dt.float32
BF16 = mybir.dt.bfloat16
U32 = mybir.dt.uint32

COMMIT_COST = 0.25

_CACHE = {}


def build_nc():
    # Bacc (not bass.Bass): its compile() legalizes sync waits — walrus
    # accepts at most one wait per instruction, and Bacc's
    # generate_event_semaphores/move_matmul_waits_to_ldweights passes
    # split excess waits onto standalone event-semaphore instructions.
    nc = bacc.Bacc("TRN2")

    x_d = nc.dram_tensor("x", [N, D], F32, kind="ExternalInput")
    e_d = nc.dram_tensor("E", [K, D], F32, kind="ExternalInput")
    # hi and lo bf16 halves of (2E)^T stacked: [2*D, K]
    et2_d = nc.dram_tensor("Et2", [2 * D, K], BF16, kind="ExternalInput")
    brep_d = nc.dram_tensor("Brep", [P, K], F32, kind="ExternalInput")

    qst_d = nc.dram_tensor("qst", [N, D], F32, kind="ExternalOutput")
    idx_d = nc.dram_tensor("idx", [P, NT], U32, kind="ExternalOutput")
    lpart_d = nc.dram_tensor("lpart", [P, NT], F32, kind="ExternalOutput")

    with tile.TileContext(nc) as tc:
        with (
            tc.tile_pool(name="persist", bufs=1) as pp,
            tc.tile_pool(name="work", bufs=3) as wp,
            tc.tile_pool(name="big", bufs=3) as bp,
            tc.tile_pool(name="ps", bufs=2, space="PSUM") as psp,
        ):
            ident = pp.tile([P, P], F32)
            make_identity(nc, ident[:])

            # One DMA for the whole split codebook keeps the matmuls' sync
            # wait count within ISA limits (one DMA completion semaphore).
            et_all = pp.tile([P, 2 * NC_CHUNKS, K], BF16)
            nc.sync.dma_start(
                et_all[:], et2_d.rearrange("(g c p) k -> p (g c) k", g=2, p=P))
            ethi = [et_all[:, c, :] for c in range(NC_CHUNKS)]
            etlo = [et_all[:, NC_CHUNKS + c, :] for c in range(NC_CHUNKS)]
            brep = pp.tile([P, K], F32)
            nc.sync.dma_start(brep[:], brep_d[:])

            idx_stage = pp.tile([P, NT], U32)
            lpart = pp.tile([P, NT], F32)

            for t in range(NT):
                r0 = t * P
                # ---- load x tile [128 rows, 512 d] ----
                xt = wp.tile([P, D], F32, tag="xt")
                nc.sync.dma_start(xt[:], x_d[r0:r0 + P, :])

                # ---- transpose to [d, rows] via PE (4 chunks) ----
                xT_ps = psp.tile([P, D], F32, tag="xT")
                for c in range(NC_CHUNKS):
                    nc.tensor.transpose(
                        xT_ps[:, c * P:(c + 1) * P],
                        xt[:, c * P:(c + 1) * P],
                        ident[:],
                    )

                # ---- bf16 split of x^T (hi + lo) ----
                xT_hi = wp.tile([P, D], BF16, tag="xThi")
                xT_lo = wp.tile([P, D], BF16, tag="xTlo")
                nc.scalar.copy(xT_hi[:], xT_ps[:])
                nc.vector.tensor_tensor(
                    out=xT_lo[:], in0=xT_ps[:], in1=xT_hi[:],
                    op=mybir.AluOpType.subtract,
                )

                # ---- A = sum(x^2) per row (ACT Square + accumulate) ----
                sq = wp.tile([P, D], F32, tag="sq")
                a_col = wp.tile([P, 1], F32, tag="acol")
                nc.scalar.activation(
                    sq[:], xt[:], mybir.ActivationFunctionType.Square,
                    accum_out=a_col[:],
                )

                # ---- C = 2 x . E^T : 3-pass bf16 split matmul ----
                c_ps = psp.tile([P, K], F32, tag="C")
                for kh in range(KH):
                    mms = []
                    for lhs, rhs in ((xT_hi, ethi), (xT_hi, etlo), (xT_lo, ethi)):
                        for c in range(NC_CHUNKS):
                            mms.append((lhs, rhs[c], c))
                    for i, (lhs, rhs_t, c) in enumerate(mms):
                        nc.tensor.matmul(
                            c_ps[:, kh * 512:(kh + 1) * 512],
                            lhsT=lhs[:, c * P:(c + 1) * P],
                            rhs=rhs_t[:, kh * 512:(kh + 1) * 512],
                            start=(i == 0),
                            stop=(i == len(mms) - 1),
                        )

                # ---- AB = fl(B + A) broadcast (DVE; gpsimd tensor_scalar
                # with an AP scalar measured ~15us/tile on HW) ----
                ab = bp.tile([P, K], F32, tag="ab")
                nc.vector.tensor_scalar_add(ab[:], brep[:], a_col[:])

                # ---- key = fl(C - AB)  (= -dist, shifted) ----
                key = bp.tile([P, K], F32, tag="key")
                nc.vector.tensor_tensor(
                    out=key[:], in0=c_ps[:], in1=ab[:],
                    op=mybir.AluOpType.subtract,
                )

                # ---- argmax (first index on ties) ----
                mx = wp.tile([P, 8], F32, tag="mx")
                mi = wp.tile([P, 8], U32, tag="mi")
                nc.vector.max(out=mx[:], in_=key[:])
                nc.vector.max_index(out=mi[:], in_max=mx[:], in_values=key[:])
                nc.gpsimd.tensor_copy(idx_stage[:, t:t + 1], mi[:, 0:1])

                # ---- gather q = E[idx] ----
                q = wp.tile([P, D], F32, tag="q")
                nc.gpsimd.indirect_dma_start(
                    out=q[:],
                    out_offset=None,
                    in_=e_d[:],
                    in_offset=bass.IndirectOffsetOnAxis(ap=mi[:, 0:1], axis=0),
                )

                # ---- diff = q - x ; qst = x + diff ; loss partial ----
                diff = wp.tile([P, D], F32, tag="diff")
                nc.gpsimd.tensor_tensor(
                    out=diff[:], in0=q[:], in1=xt[:],
                    op=mybir.AluOpType.subtract,
                )
                sq2 = wp.tile([P, D], F32, tag="sq2")
                nc.scalar.activation(
                    sq2[:], diff[:], mybir.ActivationFunctionType.Square,
                    accum_out=lpart[:, t:t + 1],
                )
                qst = wp.tile([P, D], F32, tag="qst")
                nc.gpsimd.tensor_tensor(
                    out=qst[:], in0=xt[:], in1=diff[:],
                    op=mybir.AluOpType.add,
                )
                nc.sync.dma_start(qst_d[r0:r0 + P, :], qst[:])

            nc.sync.dma_start(idx_d[:], idx_stage[:])
            nc.sync.dma_start(lpart_d[:], lpart[:])

    nc.compile()
    return nc


def _host_prep(inputs, embed_w):
    x = np.ascontiguousarray(np.asarray(inputs, dtype=np.float32))
    E = np.ascontiguousarray(np.asarray(embed_w, dtype=np.float32))
    assert x.shape == (B_DIM, T_DIM, D) and E.shape == (K, D)

    E2t = np.ascontiguousarray((2.0 * E).T)           # (512, 1024) fp32
    EtHi = E2t.astype(ml_dtypes.bfloat16)
    EtLo = (E2t - EtHi.astype(np.float32)).astype(ml_dtypes.bfloat16)
    Et2 = np.ascontiguousarray(np.concatenate([EtHi, EtLo], axis=0))
    Bsq = np.sum(E * E, axis=1, dtype=np.float32)     # (1024,)
    Brep = np.ascontiguousarray(np.broadcast_to(Bsq, (P, K)))
    return x, E, Et2, Brep


def kernel(inputs, embed_w, _trace=False):
    x, E, Et2, Brep = _host_prep(inputs, embed_w)

    if "nc" not in _CACHE:
        _CACHE["nc"] = build_nc()
    nc = _CACHE["nc"]

    in_maps = []
    for c in range(NCORES):
        in_maps.append({
            "x": np.ascontiguousarray(x[c]),
            "E": E,
            "Et2": Et2,
            "Brep": Brep,
        })

    if _trace:
        try:
            res = run_bass_kernel_spmd(
                nc, in_maps, core_ids=list(range(NCORES)), trace=True,
            )
        except Exception as e:  # tracing infra missing -> plain run
            print(f"trace run failed ({type(e).__name__}: {e}); rerunning untraced")
            res = run_bass_kernel_spmd(nc, in_maps, core_ids=list(range(NCORES)))
    else:
        res = run_bass_kernel_spmd(nc, in_maps, core_ids=list(range(NCORES)))

    qst = np.stack([res.results[c]["qst"] for c in range(NCORES)])
    qst = qst.reshape(B_DIM, T_DIM, D)

    # idx staged as [128 partitions, 32 tiles]: row 128*t + p  ->  arr[p, t]
    idx = np.stack([
        np.ascontiguousarray(res.results[c]["idx"].T).reshape(-1)
        for c in range(NCORES)
    ]).astype(np.int32)

    total = np.float64(0.0)
    for c in range(NCORES):
        total += res.results[c]["lpart"].sum(dtype=np.float64)
    loss = np.float32(COMMIT_COST * total / (B_DIM * T_DIM * D))

    if _trace:
        _CACHE["last_result"] = res
    return qst, idx, loss


# revision 22
# speedup vs baseline: 2.5802x; 1.1049x over previous
"""VQ codebook (vector-quantization) kernel for Trainium2, 8 NeuronCores.

Math (per the nn.Module reference):
    flat = inputs.reshape(-1, 512)                    # (32768, 512)
    d[n,k] = ||flat_n||^2 + ||E_k||^2 - 2 flat_n . E_k
    idx    = argmin_k d  (first index wins ties)
    q      = E[idx]
    qst    = flat + (q - flat)          # straight-through, fp32 rounding kept
    loss   = 0.25 * mean((q - flat)^2)

Sharding: data-parallel over B=8 (one batch row of 4096 tokens per core),
codebook replicated. The scalar loss is reduced on the host from per-core
partial sums (the only cross-core term).

Numerical strategy (required to match the fp32 CPU reference's argmin,
including its ~75 exact fp32 distance ties):
  - the 2*x.E term is computed as a 3-pass bf16-split matmul
    (hi*hi + hi*lo + lo*hi of x and 2E^T) accumulated in fp32 PSUM;
    error vs a true fp32 matmul is ~1e-8, far below the reference's own
    distance-grid quantization (ulp(~512) = 6.1e-5).
  - the reference's rounding structure fl(fl(A+B) - C) is replicated
    exactly: A=||x||^2 per row, B=||E||^2 per code, C=2*x.E.
  - argmin via max8/max_index on the negated key (C - AB), which returns
    the first (lowest) index among ties, matching jnp.argmin.
"""

import numpy as np
import ml_dtypes

import concourse.bacc as bacc
import concourse.bass as bass
import concourse.mybir as mybir
import concourse.tile as tile
from concourse.bass_utils import run_bass_kernel_spmd
from concourse.masks import make_identity

NCORES = 8
B_DIM, T_DIM, D, K = 8, 4096, 512, 1024
N = T_DIM            # rows per core
P = 128              # partitions
NT = N // P          # 32 row-tiles per core
NC_CHUNKS = D // P   # 4 contraction chunks
KH = K // 512        # 2 psum halves

F32 = mybir.dt.float32
BF16 = mybir.dt.bfloat16
U32 = mybir.dt.uint32

COMMIT_COST = 0.25

_CACHE = {}


def build_nc():
    # Bacc (not bass.Bass): its compile() legalizes sync waits — walrus
    # accepts at most one wait per instruction, and Bacc's
    # generate_event_semaphores/move_matmul_waits_to_ldweights passes
    # split excess waits onto standalone event-semaphore instructions.
    nc = bacc.Bacc("TRN2")

    x_d = nc.dram_tensor("x", [N, D], F32, kind="ExternalInput")
    e_d = nc.dram_tensor("E", [K, D], F32, kind="ExternalInput")
    # hi and lo bf16 halves of (2E)^T stacked: [2*D, K]
    et2_d = nc.dram_tensor("Et2", [2 * D, K], BF16, kind="ExternalInput")
    brep_d = nc.dram_tensor("Brep", [P, K], F32, kind="ExternalInput")

    qst_d = nc.dram_tensor("qst", [N, D], F32, kind="ExternalOutput")
    idx_d = nc.dram_tensor("idx", [P, NT], U32, kind="ExternalOutput")
    lpart_d = nc.dram_tensor("lpart", [P, NT], F32, kind="ExternalOutput")

    SKEW = 2  # software-pipeline depth: front(t) runs SKEW tiles ahead of back(t)

    with tile.TileContext(nc) as tc:
        with (
            tc.tile_pool(name="persist", bufs=1) as pp,
            tc.tile_pool(name="work", bufs=SKEW + 2) as wp,
            tc.tile_pool(name="big", bufs=SKEW + 1) as bp,
            tc.tile_pool(name="ps", bufs=SKEW + 1, space="PSUM") as psp,
            tc.tile_pool(name="psx", bufs=2, space="PSUM") as psxp,
        ):
            ident = pp.tile([P, P], F32)
            make_identity(nc, ident[:])

            # One DMA for the whole split codebook keeps the matmuls' sync
            # wait count within ISA limits (one DMA completion semaphore).
            et_all = pp.tile([P, 2 * NC_CHUNKS, K], BF16)
            nc.sync.dma_start(
                et_all[:], et2_d.rearrange("(g c p) k -> p (g c) k", g=2, p=P))
            ethi = [et_all[:, c, :] for c in range(NC_CHUNKS)]
            etlo = [et_all[:, NC_CHUNKS + c, :] for c in range(NC_CHUNKS)]
            brep = pp.tile([P, K], F32)
            nc.sync.dma_start(brep[:], brep_d[:])

            idx_stage = pp.tile([P, NT], U32)
            lpart = pp.tile([P, NT], F32)

            state = {}

            def front(t):
                r0 = t * P
                # ---- load x tile [128 rows, 512 d] ----
                xt = wp.tile([P, D], F32, tag="xt")
                nc.sync.dma_start(xt[:], x_d[r0:r0 + P, :])

                # ---- transpose to [d, rows] via PE (4 chunks) ----
                xT_ps = psxp.tile([P, D], F32, tag="xT")
                for c in range(NC_CHUNKS):
                    nc.tensor.transpose(
                        xT_ps[:, c * P:(c + 1) * P],
                        xt[:, c * P:(c + 1) * P],
                        ident[:],
                    )

                # ---- bf16 split of x^T (hi + lo) ----
                xT_hi = wp.tile([P, D], BF16, tag="xThi")
                xT_lo = wp.tile([P, D], BF16, tag="xTlo")
                nc.scalar.copy(xT_hi[:], xT_ps[:])
                nc.vector.tensor_tensor(
                    out=xT_lo[:], in0=xT_ps[:], in1=xT_hi[:],
                    op=mybir.AluOpType.subtract,
                )

                # ---- A = sum(x^2) per row (ACT Square + accumulate) ----
                sq = wp.tile([P, D], F32, tag="sq")
                a_col = wp.tile([P, 1], F32, tag="acol")
                nc.scalar.activation(
                    sq[:], xt[:], mybir.ActivationFunctionType.Square,
                    accum_out=a_col[:],
                )

                # ---- AB = fl(B + A) broadcast (DVE) ----
                ab = bp.tile([P, K], F32, tag="ab")
                nc.vector.tensor_scalar_add(ab[:], brep[:], a_col[:])

                # ---- C = 2 x . E^T : 3-pass bf16 split matmul ----
                c_ps = psp.tile([P, K], F32, tag="C")
                for kh in range(KH):
                    mms = []
                    for lhs, rhs in ((xT_hi, ethi), (xT_hi, etlo), (xT_lo, ethi)):
                        for c in range(NC_CHUNKS):
                            mms.append((lhs, rhs[c], c))
                    for i, (lhs, rhs_t, c) in enumerate(mms):
                        nc.tensor.matmul(
                            c_ps[:, kh * 512:(kh + 1) * 512],
                            lhsT=lhs[:, c * P:(c + 1) * P],
                            rhs=rhs_t[:, kh * 512:(kh + 1) * 512],
                            start=(i == 0),
                            stop=(i == len(mms) - 1),
                        )
                state[t] = (xt, c_ps, ab)

            def back(t):
                r0 = t * P
                xt, c_ps, ab = state.pop(t)

                # ---- key = fl(C - AB)  (= -dist, shifted) ----
                key = bp.tile([P, K], F32, tag="key")
                nc.vector.tensor_tensor(
                    out=key[:], in0=c_ps[:], in1=ab[:],
                    op=mybir.AluOpType.subtract,
                )

                # ---- argmax (first index on ties) ----
                mx = wp.tile([P, 8], F32, tag="mx")
                mi = wp.tile([P, 8], U32, tag="mi")
                nc.vector.max(out=mx[:], in_=key[:])
                nc.vector.max_index(out=mi[:], in_max=mx[:], in_values=key[:])
                nc.gpsimd.tensor_copy(idx_stage[:, t:t + 1], mi[:, 0:1])

                # ---- gather q = E[idx] ----
                q = wp.tile([P, D], F32, tag="q")
                nc.gpsimd.indirect_dma_start(
                    out=q[:],
                    out_offset=None,
                    in_=e_d[:],
                    in_offset=bass.IndirectOffsetOnAxis(ap=mi[:, 0:1], axis=0),
                )

                # ---- diff = q - x ; qst = x + diff ; loss partial ----
                diff = wp.tile([P, D], F32, tag="diff")
                nc.gpsimd.tensor_tensor(
                    out=diff[:], in0=q[:], in1=xt[:],
                    op=mybir.AluOpType.subtract,
                )
                sq2 = wp.tile([P, D], F32, tag="sq2")
                nc.scalar.activation(
                    sq2[:], diff[:], mybir.ActivationFunctionType.Square,
                    accum_out=lpart[:, t:t + 1],
                )
                qst = wp.tile([P, D], F32, tag="qst")
                nc.gpsimd.tensor_tensor(
                    out=qst[:], in0=xt[:], in1=diff[:],
                    op=mybir.AluOpType.add,
                )
                nc.sync.dma_start(qst_d[r0:r0 + P, :], qst[:])

            for i in range(NT + SKEW):
                if i < NT:
                    front(i)
                if i >= SKEW:
                    back(i - SKEW)

            nc.sync.dma_start(idx_d[:], idx_stage[:])
            nc.sync.dma_start(lpart_d[:], lpart[:])

    nc.compile()
    return nc


def _host_prep(inputs, embed_w):
    x = np.ascontiguousarray(np.asarray(inputs, dtype=np.float32))
    E = np.ascontiguousarray(np.asarray(embed_w, dtype=np.float32))
    assert x.shape == (B_DIM, T_DIM, D) and E.shape == (K, D)

    E2t = np.ascontiguousarray((2.0 * E).T)           # (512, 1024) fp32
    EtHi = E2t.astype(ml_dtypes.bfloat16)
    EtLo = (E2t - EtHi.astype(np.float32)).astype(ml_dtypes.bfloat16)
    Et2 = np.ascontiguousarray(np.concatenate([EtHi, EtLo], axis=0))
    Bsq = np.sum(E * E, axis=1, dtype=np.float32)     # (1024,)
    Brep = np.ascontiguousarray(np.broadcast_to(Bsq, (P, K)))
    return x, E, Et2, Brep


def kernel(inputs, embed_w, _trace=False):
    x, E, Et2, Brep = _host_prep(inputs, embed_w)

    if "nc" not in _CACHE:
        _CACHE["nc"] = build_nc()
    nc = _CACHE["nc"]

    in_maps = []
    for c in range(NCORES):
        in_maps.append({
            "x": np.ascontiguousarray(x[c]),
            "E": E,
            "Et2": Et2,
            "Brep": Brep,
        })

    if _trace:
        try:
            res = run_bass_kernel_spmd(
                nc, in_maps, core_ids=list(range(NCORES)), trace=True,
            )
        except Exception as e:  # tracing infra missing -> plain run
            print(f"trace run failed ({type(e).__name__}: {e}); rerunning untraced")
            res = run_bass_kernel_spmd(nc, in_maps, core_ids=list(range(NCORES)))
    else:
        res = run_bass_kernel_spmd(nc, in_maps, core_ids=list(range(NCORES)))

    qst = np.stack([res.results[c]["qst"] for c in range(NCORES)])
    qst = qst.reshape(B_DIM, T_DIM, D)

    # idx staged as [128 partitions, 32 tiles]: row 128*t + p  ->  arr[p, t]
    idx = np.stack([
        np.ascontiguousarray(res.results[c]["idx"].T).reshape(-1)
        for c in range(NCORES)
    ]).astype(np.int32)

    total = np.float64(0.0)
    for c in range(NCORES):
        total += res.results[c]["lpart"].sum(dtype=np.float64)
    loss = np.float32(COMMIT_COST * total / (B_DIM * T_DIM * D))

    if _trace:
        _CACHE["last_result"] = res
    return qst, idx, loss
